# revision 18
# baseline (speedup 1.0000x reference)
"""Trainium2 Bass kernel for the Energy Transformer problem.

Sharding: data-parallel over batch B=8 — one batch element per NeuronCore,
zero collectives.  All state stays SBUF-resident across the 12 descent steps;
only the Hopfield memory matrix (xi) is streamed from HBM during the gradient
phase.

Per-core layout convention: feature-major ("F layout") — tensors of shape
[feat, tokens] stored as SBUF [128, feat//128, tokens] with feat on partitions.

Per step (analytic gradient of the energy, derived by hand and validated
against jax.grad):
  g      = LayerNorm(x)                        (stats via ones-matmuls)
  Q,K    = Wq g, Wk g                          (both [hy,n] and [n,hy] layouts)
  per head h:
    ET[n,m] = exp(beta * q_n . k_m)            (accum_out gives s[n] for free)
    E [m,n] = exp(beta * k_m . q_n)
    aq[y,n] = sum_m K[m,y] E[m,n]              (attn-Q term, normalized by 1/s)
    ak[y,n] = sum_n' ET[n',m] (Q[n',y]/s[n'])  (attn-K term)
  hid[m,n] = Xi g ;  r = relu(hid)
  x += alpha * (Wq^T aq + Wk^T ak + Xi^T r)    (one PSUM accumulation chain)
"""

import os
import threading

import numpy as np
import ml_dtypes

import concourse.bass as bass
import concourse.mybir as mybir
import concourse.tile as tile
from concourse import bacc
from concourse.bass import ts

# ---------------------------------------------------------------- constants
B, N, D = 8, 1024, 768
H, Y = 12, 64
HY = H * Y          # 768
M = 3072
STEPS = 12
ALPHA = 0.1
BETA = 1.0 / float(np.sqrt(Y))
EPS = 1e-5

P = 128
DS = D // P         # 6  d-subtiles
NT = N // P         # 8  token tiles
NCH = N // 512      # 2  512-wide free chunks
MS = M // P         # 24 memory subtiles
NPAIR = H // 2      # 6  head pairs

F32 = mybir.dt.float32
BF16 = mybir.dt.bfloat16
AF = mybir.ActivationFunctionType
ALU = mybir.AluOpType
AX = mybir.AxisListType
ET_ = mybir.EngineType

_lock = threading.Lock()
_cache = {}


# ---------------------------------------------------------------- builder
def build_nc(steps=STEPS, loop_mode="fori"):
    """Build the per-core Bass kernel. Same NEFF runs SPMD on all 8 cores."""
    # Allow using the full 208 KiB/partition of SBUF (stale default is 192).
    try:
        from concourse import tile_utils
        tile_utils.max_sbuf_usage = 208 * 1024
    except Exception:
        pass

    nc = bacc.Bacc("TRN2", target_bir_lowering=False, debug=False)

    # DRAM I/O (per core). Weight tensors are pre-transposed/scaled on host.
    x_d = nc.dram_tensor("x", [D, N], F32, kind="ExternalInput")
    wqT_d = nc.dram_tensor("wqT", [D, HY], BF16, kind="ExternalInput")
    wkT_d = nc.dram_tensor("wkT", [D, HY], BF16, kind="ExternalInput")
    wqF_d = nc.dram_tensor("wqF", [HY, D], BF16, kind="ExternalInput")
    wkF_d = nc.dram_tensor("wkF", [HY, D], BF16, kind="ExternalInput")
    xiT_d = nc.dram_tensor("xiT", [D, M], BF16, kind="ExternalInput")
    xiS_d = nc.dram_tensor("xiS", [M, D], BF16, kind="ExternalInput")
    gam_d = nc.dram_tensor("gamma", [D], F32, kind="ExternalInput")
    bet_d = nc.dram_tensor("beta", [D], F32, kind="ExternalInput")
    xo_d = nc.dram_tensor("xout", [D, N], F32, kind="ExternalOutput")

    # Persistent SBUF state.
    xs = nc.alloc_sbuf_tensor("xs", [P, DS, N], F32).ap()
    gs = nc.alloc_sbuf_tensor("gs", [P, DS, N], BF16).ap()
    wqTs = nc.alloc_sbuf_tensor("wqTs", [P, DS, HY], BF16).ap()
    wkTs = nc.alloc_sbuf_tensor("wkTs", [P, DS, HY], BF16).ap()
    wqFs = nc.alloc_sbuf_tensor("wqFs", [P, DS, D], BF16).ap()
    wkFs = nc.alloc_sbuf_tensor("wkFs", [P, DS, D], BF16).ap()
    qtok = nc.alloc_sbuf_tensor("qtok", [P, NT, HY], BF16).ap()
    ktok = nc.alloc_sbuf_tensor("ktok", [P, NT, HY], BF16).ap()
    aq = nc.alloc_sbuf_tensor("aq", [P, DS, N], BF16).ap()
    ak = nc.alloc_sbuf_tensor("ak", [P, DS, N], BF16).ap()
    gam_s = nc.alloc_sbuf_tensor("gam_s", [P, DS], F32).ap()
    bet_s = nc.alloc_sbuf_tensor("bet_s", [P, DS], F32).ap()
    ones_c = nc.alloc_sbuf_tensor("ones_c", [P, 1], F32).ap()   # lhsT for sums
    ones_r = nc.alloc_sbuf_tensor("ones_r", [1, P], F32).ap()   # lhsT for bcast
    eps_c = nc.alloc_sbuf_tensor("eps_c", [1, 1], F32).ap()
    rs_all = nc.alloc_sbuf_tensor("rs_all", [P, H, NT], F32).ap()

    from contextlib import ExitStack
    with tile.TileContext(nc) as tc, ExitStack() as stack:
        sb = stack.enter_context(tc.tile_pool(name="sb", bufs=2))
        psum = stack.enter_context(
            tc.tile_pool(name="psum", bufs=4, space="PSUM"))

        def pbig(name):
            return psum.tile([P, 1024], F32, tag="pb", name=name)

        # ---- one-time loads
        nc.gpsimd.memset(ones_c[:], 1.0)
        nc.gpsimd.memset(ones_r[:], 1.0)
        nc.gpsimd.memset(eps_c[:], EPS)
        nc.sync.dma_start(xs[:], x_d.ap().rearrange("(o p) n -> p o n", p=P))
        nc.sync.dma_start(wqTs[:], wqT_d.ap().rearrange("(o p) h -> p o h", p=P))
        nc.sync.dma_start(wkTs[:], wkT_d.ap().rearrange("(o p) h -> p o h", p=P))
        nc.sync.dma_start(wqFs[:], wqF_d.ap().rearrange("(o p) d -> p o d", p=P))
        nc.sync.dma_start(wkFs[:], wkF_d.ap().rearrange("(o p) d -> p o d", p=P))
        with nc.allow_non_contiguous_dma(reason="tiny 768-elem transposes"):
            nc.sync.dma_start(gam_s[:],
                              gam_d.ap().rearrange("(o p) -> p o", p=P))
            nc.sync.dma_start(bet_s[:],
                              bet_d.ap().rearrange("(o p) -> p o", p=P))

        xiT_v = xiT_d.ap().rearrange("(o p) m -> p o m", p=P)
        xiS_v = xiS_d.ap().rearrange("(o p) d -> p o d", p=P)

        def emit_step():
            # ---------------- Phase A: LayerNorm -> gs (bf16)
            for c in range(NCH):
                nsl = ts(c, 512)
                stat = pbig("stat")
                s1p = stat[:1, 0:512]
                s2p = stat[:1, 512:1024]
                for d in range(DS):
                    x2t = sb.tile([P, 512], F32, tag="x2", bufs=1, name="x2t")
                    nc.vector.tensor_tensor(
                        x2t[:], xs[:, d, nsl], xs[:, d, nsl], ALU.mult)
                    nc.tensor.matmul(
                        s1p, ones_c[:], xs[:, d, nsl],
                        start=(d == 0), stop=(d == DS - 1),
                        skip_group_check=True)
                    nc.tensor.matmul(
                        s2p, ones_c[:], x2t[:],
                        start=(d == 0), stop=(d == DS - 1),
                        skip_group_check=True)
                mu = sb.tile([1, 512], F32, tag="mu", name="mu")
                nc.vector.tensor_scalar_mul(mu[:], s1p, 1.0 / D)
                var = sb.tile([1, 512], F32, tag="var", name="var")
                nc.vector.tensor_scalar_mul(var[:], s2p, 1.0 / D)
                musq = sb.tile([1, 512], F32, tag="musq", name="musq")
                nc.vector.tensor_tensor(musq[:], mu[:], mu[:], ALU.mult)
                nc.vector.tensor_tensor(var[:], var[:], musq[:], ALU.subtract)
                # rstd = exp(-0.5*ln(var+eps))
                rstd = sb.tile([1, 512], F32, tag="rstd", name="rstd")
                nc.scalar.activation(rstd[:], var[:], AF.Ln, bias=eps_c[:])
                nc.scalar.activation(rstd[:], rstd[:], AF.Exp, scale=-0.5)
                # replicate mu/rstd across partitions via K=1 matmul
                rep = pbig("rep")
                mur = rep[:, 0:512]
                rsr = rep[:, 512:1024]
                nc.tensor.matmul(mur, ones_r[:1, :], mu[:],
                                 start=True, stop=True, skip_group_check=True)
                nc.tensor.matmul(rsr, ones_r[:1, :], rstd[:],
                                 start=True, stop=True, skip_group_check=True)
                for d in range(DS):
                    tt = sb.tile([P, 512], F32, tag="lnt", bufs=1, name="tt")
                    nc.vector.tensor_tensor(
                        tt[:], xs[:, d, nsl], mur, ALU.subtract)
                    nc.vector.tensor_tensor(tt[:], tt[:], rsr, ALU.mult)
                    nc.vector.tensor_scalar(
                        gs[:, d, nsl], tt[:],
                        gam_s[:, d:d + 1], bet_s[:, d:d + 1],
                        ALU.mult, ALU.add)

            # ---------------- Phase B: token-layout Q/K projections
            for t in range(NT):
                qtp = pbig("qtp")
                ktp = pbig("ktp")
                for c0, cw in ((0, 512), (512, 256)):
                    for d in range(DS):
                        nc.tensor.matmul(
                            qtp[:, c0:c0 + cw], gs[:, d, ts(t, P)],
                            wqTs[:, d, c0:c0 + cw],
                            start=(d == 0), stop=(d == DS - 1),
                            skip_group_check=True)
                        nc.tensor.matmul(
                            ktp[:, c0:c0 + cw], gs[:, d, ts(t, P)],
                            wkTs[:, d, c0:c0 + cw],
                            start=(d == 0), stop=(d == DS - 1),
                            skip_group_check=True)
                nc.scalar.copy(qtok[:, t, :], qtp[:, :HY])
                nc.scalar.copy(ktok[:, t, :], ktp[:, :HY])

            # ---------------- Phase C: attention, per head pair
            for hp in range(NPAIR):
                ha, hb = 2 * hp, 2 * hp + 1
                # F-layout Q/K for this pair's 128 hy rows
                qf = sb.tile([P, N], BF16, tag="qf", bufs=2, name="qf")
                kf = sb.tile([P, N], BF16, tag="kf", bufs=2, name="kf")
                qp = pbig("qp")
                kp = pbig("kp")
                for c in range(NCH):
                    nsl = ts(c, 512)
                    for d in range(DS):
                        nc.tensor.matmul(
                            qp[:, nsl], wqTs[:, d, ts(hp, P)], gs[:, d, nsl],
                            start=(d == 0), stop=(d == DS - 1),
                            skip_group_check=True)
                        nc.tensor.matmul(
                            kp[:, nsl], wkTs[:, d, ts(hp, P)], gs[:, d, nsl],
                            start=(d == 0), stop=(d == DS - 1),
                            skip_group_check=True)
                nc.scalar.copy(qf[:], qp[:])
                nc.scalar.copy(kf[:], kp[:])

                # ET pass (both heads, row-packed K=64 matmuls):
                # ET[n,m] = exp(beta q_n.k_m); accum_out -> s[n]
                # "ebig" slots are reused by the E pass after attnK drains ET.
                eta = sb.tile([P, NT, N], BF16, tag="ebig", bufs=3, name="eta")
                etb = sb.tile([P, NT, N], BF16, tag="ebig", bufs=3, name="etb")
                sca = sb.tile([P, NT], F32, tag="sca", name="sca")
                scb = sb.tile([P, NT], F32, tag="scb", name="scb")
                for t in range(NT):
                    pa = pbig("pa")
                    pb = pbig("pb")
                    for c in range(NCH):
                        msl = ts(c, 512)
                        nc.tensor.matmul(
                            pa[:, msl], qf[0:64, ts(t, P)], kf[0:64, msl],
                            start=True, stop=True, skip_group_check=True)
                        nc.tensor.matmul(
                            pb[:, msl], qf[64:128, ts(t, P)], kf[64:128, msl],
                            start=True, stop=True, skip_group_check=True)
                    nc.scalar.activation(
                        eta[:, t, :], pa[:], AF.Exp, scale=BETA,
                        accum_out=sca[:, t:t + 1])
                    nc.scalar.activation(
                        etb[:, t, :], pb[:], AF.Exp, scale=BETA,
                        accum_out=scb[:, t:t + 1])
                # s -> 1/s ; Q' = Q * (1/s)  (per-token scale, partition dim)
                for h, sc in ((ha, sca), (hb, scb)):
                    nc.vector.reciprocal(rs_all[:, h, :], sc[:])
                    nc.vector.tensor_tensor(
                        qtok[:, :, ts(h, Y)], qtok[:, :, ts(h, Y)],
                        rs_all[:, h, :, None].to_broadcast([P, NT, Y]),
                        ALU.mult)

                # attnK: ak[y,m] = sum_n ET[n,m] Q'[n,y]  (col-packed heads)
                akp = pbig("akp")
                for c in range(NCH):
                    msl = ts(c, 512)
                    for t in range(NT):
                        nc.tensor.matmul(
                            akp[0:64, msl], qtok[:, t, ts(ha, Y)],
                            eta[:, t, msl],
                            start=(t == 0), stop=(t == NT - 1),
                            skip_group_check=True)
                        nc.tensor.matmul(
                            akp[64:128, msl], qtok[:, t, ts(hb, Y)],
                            etb[:, t, msl],
                            start=(t == 0), stop=(t == NT - 1),
                            tile_position=(0, 64), skip_group_check=True)
                nc.vector.tensor_copy(ak[:, hp, :], akp[:])

                # E pass: E[m,n] = exp(beta k_m.q_n)  (reuses the ET slots)
                ea = sb.tile([P, NT, N], BF16, tag="ebig", bufs=3, name="ea")
                eb = sb.tile([P, NT, N], BF16, tag="ebig", bufs=3, name="eb")
                for t in range(NT):
                    pa = pbig("pa")
                    pb = pbig("pb")
                    for c in range(NCH):
                        nsl = ts(c, 512)
                        nc.tensor.matmul(
                            pa[:, nsl], kf[0:64, ts(t, P)], qf[0:64, nsl],
                            start=True, stop=True, skip_group_check=True)
                        nc.tensor.matmul(
                            pb[:, nsl], kf[64:128, ts(t, P)], qf[64:128, nsl],
                            start=True, stop=True, skip_group_check=True)
                    nc.scalar.activation(ea[:, t, :], pa[:], AF.Exp,
                                         scale=BETA)
                    nc.scalar.activation(eb[:, t, :], pb[:], AF.Exp,
                                         scale=BETA)

                # attnQ: aq[y,n] = sum_m K[m,y] E[m,n]   (col-packed heads)
                aqp = pbig("aqp")
                for c in range(NCH):
                    nsl = ts(c, 512)
                    for t in range(NT):
                        nc.tensor.matmul(
                            aqp[0:64, nsl], ktok[:, t, ts(ha, Y)],
                            ea[:, t, nsl],
                            start=(t == 0), stop=(t == NT - 1),
                            skip_group_check=True)
                        nc.tensor.matmul(
                            aqp[64:128, nsl], ktok[:, t, ts(hb, Y)],
                            eb[:, t, nsl],
                            start=(t == 0), stop=(t == NT - 1),
                            tile_position=(0, 64), skip_group_check=True)
                nc.vector.tensor_copy(aq[:, hp, :], aqp[:])

            # aq normalization: aq[y,n] *= 1/s_h[n]  (n on free axis).
            # rsf holds 1/s in (p,t)-permuted column order so the transposing
            # DMA is contiguous; AP views undo the permutation downstream.
            for h in range(H):
                hp, off = h // 2, 64 * (h % 2)
                rsf = sb.tile([1, N], F32, tag="rsf", bufs=1, name="rsf")
                nc.sync.dma_start(rsf[:], rs_all[:, h, :])
                rsv = rsf.rearrange("q (p t) -> q p t", t=NT)
                rrep = pbig("rrep")
                for c in range(NCH):
                    nsl = ts(c, 512)
                    rhs = rsv[:, :, 4 * c:4 * c + 4]
                    if off == 0:
                        nc.tensor.matmul(
                            rrep[0:64, nsl], ones_r[:1, :64], rhs,
                            start=True, stop=True, skip_group_check=True)
                    else:
                        nc.tensor.matmul(
                            rrep[64:128, nsl], ones_r[:1, :64], rhs,
                            start=True, stop=True, tile_position=(0, 64),
                            skip_group_check=True)
                    # rrep columns are (p, t)-ordered; view aq to match.
                    nc.vector.tensor_tensor(
                        aq[off:off + 64, hp, nsl]
                        .rearrange("y (t p) -> y p t", p=P),
                        aq[off:off + 64, hp, nsl]
                        .rearrange("y (t p) -> y p t", p=P),
                        rrep[off:off + 64, nsl]
                        .rearrange("y (p t) -> y p t", t=4),
                        ALU.mult)

            # ---------------- Phase D: gradient accumulation + x update
            for c in range(NCH):
                nsl = ts(c, 512)
                gbig = [pbig(f"gb{i}") for i in range(DS // 2)]
                gps = [gbig[d // 2][:, (d % 2) * 512:(d % 2) * 512 + 512]
                       for d in range(DS)]
                hbig = pbig("hbig")
                for dt in range(DS):
                    for s_ in range(DS):
                        nc.tensor.matmul(
                            gps[dt], wqFs[:, s_, ts(dt, P)], aq[:, s_, nsl],
                            start=(s_ == 0), stop=False, skip_group_check=True)
                    for s_ in range(DS):
                        nc.tensor.matmul(
                            gps[dt], wkFs[:, s_, ts(dt, P)], ak[:, s_, nsl],
                            start=False, stop=False, skip_group_check=True)
                for msp in range(MS // 2):
                    xit = sb.tile([P, DS, 2 * P], BF16, tag="xit", bufs=2,
                                  name="xit")
                    nc.sync.dma_start(xit[:], xiT_v[:, :, ts(msp, 2 * P)])
                    xis = sb.tile([P, 2, D], BF16, tag="xis", bufs=2,
                                  name="xis")
                    nc.sync.dma_start(xis[:], xiS_v[:, 2 * msp:2 * msp + 2, :])
                    for j in range(2):
                        ms = 2 * msp + j
                        hp_ = hbig[:, j * 512:j * 512 + 512]
                        for d in range(DS):
                            nc.tensor.matmul(
                                hp_, xit[:, d, ts(j, P)], gs[:, d, nsl],
                                start=(d == 0), stop=(d == DS - 1),
                                skip_group_check=True)
                        rt = sb.tile([P, 512], BF16, tag="rt", bufs=2,
                                     name="rt")
                        nc.scalar.activation(rt[:], hp_, AF.Relu)
                        for dt in range(DS):
                            nc.tensor.matmul(
                                gps[dt], xis[:, j, ts(dt, P)], rt[:],
                                start=False, stop=(ms == MS - 1),
                                skip_group_check=True)
                for dt in range(DS):
                    nc.vector.tensor_tensor(
                        xs[:, dt, nsl], xs[:, dt, nsl], gps[dt], ALU.add)

        if loop_mode == "fori" and steps > 1:
            # Final step unrolled: reads of state written inside a For_i from
            # after the loop are not dependency-tracked (observed to race), so
            # keep the loop-exit consumer chain in straight-line code.
            with tc.For_i(0, steps - 1, 1,
                          hint_engines=(ET_.PE, ET_.Activation, ET_.DVE,
                                        ET_.SP, ET_.Pool)):
                emit_step()
            emit_step()
        else:
            for _ in range(steps):
                emit_step()

        nc.sync.dma_start(
            xo_d.ap().rearrange("(o p) n -> p o n", p=P), xs[:])

    nc.compile()
    return nc


# ---------------------------------------------------------------- host side
def _prep_shared(ln_gamma, ln_beta, wq, wk, xi):
    bf = ml_dtypes.bfloat16
    wq_f = np.ascontiguousarray(wq.reshape(HY, D))
    wk_f = np.ascontiguousarray(wk.reshape(HY, D))
    return {
        "wqT": np.ascontiguousarray(wq_f.T).astype(bf),
        "wkT": np.ascontiguousarray(wk_f.T).astype(bf),
        "wqF": (ALPHA * wq_f).astype(bf),
        "wkF": (ALPHA * wk_f).astype(bf),
        "xiT": np.ascontiguousarray(xi.T).astype(bf),
        "xiS": (ALPHA * xi).astype(bf),
        "gamma": np.ascontiguousarray(ln_gamma, dtype=np.float32),
        "beta": np.ascontiguousarray(ln_beta, dtype=np.float32),
    }


def make_in_maps(x, ln_gamma, ln_beta, wq, wk, xi):
    shared = _prep_shared(np.asarray(ln_gamma), np.asarray(ln_beta),
                          np.asarray(wq), np.asarray(wk), np.asarray(xi))
    x = np.asarray(x, dtype=np.float32)
    maps = []
    for b in range(B):
        m = dict(shared)
        m["x"] = np.ascontiguousarray(x[b].T)
        maps.append(m)
    return maps


def get_executor(steps=STEPS, loop_mode="fori"):
    """Build+compile once; return (nc, run_fn). run_fn(in_maps) -> results
    list; repeated calls reuse the compiled PJRT executable."""
    key = (steps, loop_mode)
    with _lock:
        if key in _cache:
            return _cache[key]
    nc = build_nc(steps, loop_mode)

    import jax
    from jax.sharding import Mesh, PartitionSpec
    from jax.experimental.shard_map import shard_map
    from concourse import bass2jax

    bass2jax.install_neuronx_cc_hook()

    in_names, out_names, out_avals, zero_outs = [], [], [], []
    for alloc in nc.m.functions[0].allocations:
        if not isinstance(alloc, mybir.MemoryLocationSet):
            continue
        name = alloc.memorylocations[0].name
        if alloc.kind == "ExternalInput":
            in_names.append(name)
        elif alloc.kind == "ExternalOutput":
            out_names.append(name)
            shape = tuple(alloc.tensor_shape)
            dtype = mybir.dt.np(alloc.dtype)
            out_avals.append(jax.core.ShapedArray(shape, dtype))
            zero_outs.append(np.zeros(shape, dtype))
    partition_name = (nc.partition_id_tensor.name
                      if nc.partition_id_tensor else None)
    if partition_name is not None and partition_name in in_names:
        in_names.remove(partition_name)
    n_params = len(in_names)
    n_outs = len(out_avals)
    all_names = in_names + out_names
    if partition_name is not None:
        all_names = all_names + [partition_name]

    def _body(*args):
        operands = list(args)
        if partition_name is not None:
            operands.append(bass2jax.partition_id_tensor())
        outs = bass2jax._bass_exec_p.bind(
            *operands,
            out_avals=tuple(out_avals),
            in_names=tuple(all_names),
            out_names=tuple(out_names),
            lowering_input_output_aliases=(),
            sim_require_finite=True,
            sim_require_nnan=True,
            nc=nc,
        )
        return tuple(outs)

    devices = jax.devices()[:B]
    mesh = Mesh(np.asarray(devices), ("core",))
    sharded = jax.jit(
        shard_map(_body, mesh=mesh,
                  in_specs=(PartitionSpec("core"),) * (n_params + n_outs),
                  out_specs=(PartitionSpec("core"),) * n_outs,
                  check_rep=False),
        keep_unused=True,
    )

    def run(in_maps):
        per_core = [[np.asarray(m[nm]) for nm in in_names] for m in in_maps]
        concat_in = [
            np.concatenate([per_core[c][i] for c in range(B)], axis=0)
            for i in range(n_params)
        ]
        concat_zeros = [
            np.zeros((B * z.shape[0], *z.shape[1:]), z.dtype)
            for z in zero_outs
        ]
        out_arrs = sharded(*concat_in, *concat_zeros)
        out_arrs = [np.asarray(a) for a in out_arrs]
        return [
            {nm: out_arrs[i].reshape(B, *out_avals[i].shape)[c]
             for i, nm in enumerate(out_names)}
            for c in range(B)
        ]

    with _lock:
        _cache[key] = (nc, run)
    return nc, run


def kernel(x, ln_gamma, ln_beta, wq, wk, xi):
    _, run = get_executor()
    in_maps = make_in_maps(x, ln_gamma, ln_beta, wq, wk, xi)
    results = run(in_maps)
    out = np.stack([results[b]["xout"].T for b in range(B)])
    return np.ascontiguousarray(out, dtype=np.float32)


# revision 22
# speedup vs baseline: 36.9302x; 36.9302x over previous
"""Trainium2 Bass kernel for the Energy Transformer problem.

Sharding: data-parallel over batch B=8 — one batch element per NeuronCore,
zero collectives.  All state stays SBUF-resident across the 12 descent steps;
only the Hopfield memory matrix (xi) is streamed from HBM during the gradient
phase.

Per-core layout convention: feature-major ("F layout") — tensors of shape
[feat, tokens] stored as SBUF [128, feat//128, tokens] with feat on partitions.

Per step (analytic gradient of the energy, derived by hand and validated
against jax.grad):
  g      = LayerNorm(x)                        (stats via ones-matmuls)
  Q,K    = Wq g, Wk g                          (both [hy,n] and [n,hy] layouts)
  per head h:
    ET[n,m] = exp(beta * q_n . k_m)            (accum_out gives s[n] for free)
    E [m,n] = exp(beta * k_m . q_n)
    aq[y,n] = sum_m K[m,y] E[m,n]              (attn-Q term, normalized by 1/s)
    ak[y,n] = sum_n' ET[n',m] (Q[n',y]/s[n'])  (attn-K term)
  hid[m,n] = Xi g ;  r = relu(hid)
  x += alpha * (Wq^T aq + Wk^T ak + Xi^T r)    (one PSUM accumulation chain)
"""

import os
import threading

import numpy as np
import ml_dtypes

import concourse.bass as bass
import concourse.mybir as mybir
import concourse.tile as tile
from concourse import bacc
from concourse.bass import ts

# ---------------------------------------------------------------- constants
B, N, D = 8, 1024, 768
H, Y = 12, 64
HY = H * Y          # 768
M = 3072
STEPS = 12
ALPHA = 0.1
BETA = 1.0 / float(np.sqrt(Y))
EPS = 1e-5

P = 128
DS = D // P         # 6  d-subtiles
NT = N // P         # 8  token tiles
NCH = N // 512      # 2  512-wide free chunks
MS = M // P         # 24 memory subtiles
NPAIR = H // 2      # 6  head pairs

F32 = mybir.dt.float32
BF16 = mybir.dt.bfloat16
AF = mybir.ActivationFunctionType
ALU = mybir.AluOpType
AX = mybir.AxisListType
ET_ = mybir.EngineType

_lock = threading.Lock()
_cache = {}


# ---------------------------------------------------------------- builder
def build_nc(steps=STEPS, loop_mode="fori"):
    """Build the per-core Bass kernel. Same NEFF runs SPMD on all 8 cores."""
    # Allow using the full 208 KiB/partition of SBUF (stale default is 192).
    try:
        from concourse import tile_utils
        tile_utils.max_sbuf_usage = 208 * 1024
    except Exception:
        pass

    nc = bacc.Bacc("TRN2", target_bir_lowering=False, debug=False)

    # DRAM I/O (per core). Weight tensors are pre-transposed/scaled on host.
    x_d = nc.dram_tensor("x", [D, N], F32, kind="ExternalInput")
    wqT_d = nc.dram_tensor("wqT", [D, HY], BF16, kind="ExternalInput")
    wkT_d = nc.dram_tensor("wkT", [D, HY], BF16, kind="ExternalInput")
    wqF_d = nc.dram_tensor("wqF", [HY, D], BF16, kind="ExternalInput")
    wkF_d = nc.dram_tensor("wkF", [HY, D], BF16, kind="ExternalInput")
    xiT_d = nc.dram_tensor("xiT", [D, M], BF16, kind="ExternalInput")
    xiS_d = nc.dram_tensor("xiS", [M, D], BF16, kind="ExternalInput")
    gam_d = nc.dram_tensor("gamma", [D], F32, kind="ExternalInput")
    bet_d = nc.dram_tensor("beta", [D], F32, kind="ExternalInput")
    xo_d = nc.dram_tensor("xout", [D, N], F32, kind="ExternalOutput")

    # Persistent SBUF state.
    xs = nc.alloc_sbuf_tensor("xs", [P, DS, N], F32).ap()
    gs = nc.alloc_sbuf_tensor("gs", [P, DS, N], BF16).ap()
    wqTs = nc.alloc_sbuf_tensor("wqTs", [P, DS, HY], BF16).ap()
    wkTs = nc.alloc_sbuf_tensor("wkTs", [P, DS, HY], BF16).ap()
    wqFs = nc.alloc_sbuf_tensor("wqFs", [P, DS, D], BF16).ap()
    wkFs = nc.alloc_sbuf_tensor("wkFs", [P, DS, D], BF16).ap()
    qtok = nc.alloc_sbuf_tensor("qtok", [P, NT, HY], BF16).ap()
    ktok = nc.alloc_sbuf_tensor("ktok", [P, NT, HY], BF16).ap()
    aq = nc.alloc_sbuf_tensor("aq", [P, DS, N], BF16).ap()
    ak = nc.alloc_sbuf_tensor("ak", [P, DS, N], BF16).ap()
    gam_s = nc.alloc_sbuf_tensor("gam_s", [P, DS], F32).ap()
    bet_s = nc.alloc_sbuf_tensor("bet_s", [P, DS], F32).ap()
    ones_c = nc.alloc_sbuf_tensor("ones_c", [P, 1], F32).ap()   # lhsT for sums
    ones_r = nc.alloc_sbuf_tensor("ones_r", [1, P], F32).ap()   # lhsT for bcast
    eps_c = nc.alloc_sbuf_tensor("eps_c", [1, 1], F32).ap()
    rs_all = nc.alloc_sbuf_tensor("rs_all", [P, H, NT], F32).ap()

    from contextlib import ExitStack
    with tile.TileContext(nc) as tc, ExitStack() as stack:
        sb = stack.enter_context(tc.tile_pool(name="sb", bufs=2))
        psum = stack.enter_context(
            tc.tile_pool(name="psum", bufs=4, space="PSUM"))

        def pbig(name):
            return psum.tile([P, 1024], F32, tag="pb", name=name)

        # ---- one-time loads
        nc.gpsimd.memset(ones_c[:], 1.0)
        nc.gpsimd.memset(ones_r[:], 1.0)
        nc.gpsimd.memset(eps_c[:], EPS)
        nc.sync.dma_start(xs[:], x_d.ap().rearrange("(o p) n -> p o n", p=P))
        nc.sync.dma_start(wqTs[:], wqT_d.ap().rearrange("(o p) h -> p o h", p=P))
        nc.sync.dma_start(wkTs[:], wkT_d.ap().rearrange("(o p) h -> p o h", p=P))
        nc.sync.dma_start(wqFs[:], wqF_d.ap().rearrange("(o p) d -> p o d", p=P))
        nc.sync.dma_start(wkFs[:], wkF_d.ap().rearrange("(o p) d -> p o d", p=P))
        with nc.allow_non_contiguous_dma(reason="tiny 768-elem transposes"):
            nc.sync.dma_start(gam_s[:],
                              gam_d.ap().rearrange("(o p) -> p o", p=P))
            nc.sync.dma_start(bet_s[:],
                              bet_d.ap().rearrange("(o p) -> p o", p=P))

        xiT_v = xiT_d.ap().rearrange("(o p) m -> p o m", p=P)
        xiS_v = xiS_d.ap().rearrange("(o p) d -> p o d", p=P)

        def emit_step():
            # ---------------- Phase A: LayerNorm -> gs (bf16)
            for c in range(NCH):
                nsl = ts(c, 512)
                stat = pbig("stat")
                s1p = stat[:1, 0:512]
                s2p = stat[:1, 512:1024]
                for d in range(DS):
                    x2t = sb.tile([P, 512], F32, tag="x2", bufs=1, name="x2t")
                    nc.vector.tensor_tensor(
                        x2t[:], xs[:, d, nsl], xs[:, d, nsl], ALU.mult)
                    nc.tensor.matmul(
                        s1p, ones_c[:], xs[:, d, nsl],
                        start=(d == 0), stop=(d == DS - 1),
                        skip_group_check=True)
                    nc.tensor.matmul(
                        s2p, ones_c[:], x2t[:],
                        start=(d == 0), stop=(d == DS - 1),
                        skip_group_check=True)
                mu = sb.tile([1, 512], F32, tag="mu", bufs=1, name="mu")
                nc.vector.tensor_scalar_mul(mu[:], s1p, 1.0 / D)
                var = sb.tile([1, 512], F32, tag="var", bufs=1, name="var")
                nc.vector.tensor_scalar_mul(var[:], s2p, 1.0 / D)
                musq = sb.tile([1, 512], F32, tag="musq", bufs=1, name="musq")
                nc.vector.tensor_tensor(musq[:], mu[:], mu[:], ALU.mult)
                nc.vector.tensor_tensor(var[:], var[:], musq[:], ALU.subtract)
                # rstd = exp(-0.5*ln(var+eps))
                rstd = sb.tile([1, 512], F32, tag="rstd", bufs=1, name="rstd")
                nc.scalar.activation(rstd[:], var[:], AF.Ln, bias=eps_c[:])
                nc.scalar.activation(rstd[:], rstd[:], AF.Exp, scale=-0.5)
                # replicate mu/rstd across partitions via K=1 matmul
                rep = pbig("rep")
                mur = rep[:, 0:512]
                rsr = rep[:, 512:1024]
                nc.tensor.matmul(mur, ones_r[:1, :], mu[:],
                                 start=True, stop=True, skip_group_check=True)
                nc.tensor.matmul(rsr, ones_r[:1, :], rstd[:],
                                 start=True, stop=True, skip_group_check=True)
                for d in range(DS):
                    tt = sb.tile([P, 512], F32, tag="lnt", bufs=1, name="tt")
                    nc.vector.tensor_tensor(
                        tt[:], xs[:, d, nsl], mur, ALU.subtract)
                    nc.vector.tensor_tensor(tt[:], tt[:], rsr, ALU.mult)
                    nc.vector.tensor_scalar(
                        gs[:, d, nsl], tt[:],
                        gam_s[:, d:d + 1], bet_s[:, d:d + 1],
                        ALU.mult, ALU.add)

            # ---------------- Phase B: token-layout Q/K projections
            for t in range(NT):
                qtp = pbig("qtp")
                ktp = pbig("ktp")
                for c0, cw in ((0, 512), (512, 256)):
                    for d in range(DS):
                        nc.tensor.matmul(
                            qtp[:, c0:c0 + cw], gs[:, d, ts(t, P)],
                            wqTs[:, d, c0:c0 + cw],
                            start=(d == 0), stop=(d == DS - 1),
                            skip_group_check=True)
                        nc.tensor.matmul(
                            ktp[:, c0:c0 + cw], gs[:, d, ts(t, P)],
                            wkTs[:, d, c0:c0 + cw],
                            start=(d == 0), stop=(d == DS - 1),
                            skip_group_check=True)
                nc.scalar.copy(qtok[:, t, :], qtp[:, :HY])
                nc.scalar.copy(ktok[:, t, :], ktp[:, :HY])

            # ---------------- Phase C: attention, per head pair
            for hp in range(NPAIR):
                ha, hb = 2 * hp, 2 * hp + 1
                # F-layout Q/K for this pair's 128 hy rows
                qf = sb.tile([P, N], BF16, tag="qf", bufs=2, name="qf")
                kf = sb.tile([P, N], BF16, tag="kf", bufs=2, name="kf")
                qp = pbig("qp")
                kp = pbig("kp")
                for c in range(NCH):
                    nsl = ts(c, 512)
                    for d in range(DS):
                        nc.tensor.matmul(
                            qp[:, nsl], wqTs[:, d, ts(hp, P)], gs[:, d, nsl],
                            start=(d == 0), stop=(d == DS - 1),
                            skip_group_check=True)
                        nc.tensor.matmul(
                            kp[:, nsl], wkTs[:, d, ts(hp, P)], gs[:, d, nsl],
                            start=(d == 0), stop=(d == DS - 1),
                            skip_group_check=True)
                nc.scalar.copy(qf[:], qp[:])
                nc.scalar.copy(kf[:], kp[:])

                # ET pass (both heads, row-packed K=64 matmuls):
                # ET[n,m] = exp(beta q_n.k_m); accum_out -> s[n]
                # "ebig" slots are reused by the E pass after attnK drains ET.
                eta = sb.tile([P, NT, N], BF16, tag="ebig", bufs=3, name="eta")
                etb = sb.tile([P, NT, N], BF16, tag="ebig", bufs=3, name="etb")
                sca = sb.tile([P, NT], F32, tag="sca", name="sca")
                scb = sb.tile([P, NT], F32, tag="scb", name="scb")
                for t in range(NT):
                    pa = pbig("pa")
                    pb = pbig("pb")
                    for c in range(NCH):
                        msl = ts(c, 512)
                        nc.tensor.matmul(
                            pa[:, msl], qf[0:64, ts(t, P)], kf[0:64, msl],
                            start=True, stop=True, skip_group_check=True)
                        nc.tensor.matmul(
                            pb[:, msl], qf[64:128, ts(t, P)], kf[64:128, msl],
                            start=True, stop=True, skip_group_check=True)
                    nc.scalar.activation(
                        eta[:, t, :], pa[:], AF.Exp, scale=BETA,
                        accum_out=sca[:, t:t + 1])
                    nc.scalar.activation(
                        etb[:, t, :], pb[:], AF.Exp, scale=BETA,
                        accum_out=scb[:, t:t + 1])
                # s -> 1/s ; Q' = Q * (1/s)  (per-token scale, partition dim)
                for h, sc in ((ha, sca), (hb, scb)):
                    nc.vector.reciprocal(rs_all[:, h, :], sc[:])
                    nc.vector.tensor_tensor(
                        qtok[:, :, ts(h, Y)], qtok[:, :, ts(h, Y)],
                        rs_all[:, h, :, None].to_broadcast([P, NT, Y]),
                        ALU.mult)

                # attnK: ak[y,m] = sum_n ET[n,m] Q'[n,y]  (col-packed heads)
                akp = pbig("akp")
                for c in range(NCH):
                    msl = ts(c, 512)
                    for t in range(NT):
                        nc.tensor.matmul(
                            akp[0:64, msl], qtok[:, t, ts(ha, Y)],
                            eta[:, t, msl],
                            start=(t == 0), stop=(t == NT - 1),
                            skip_group_check=True)
                        nc.tensor.matmul(
                            akp[64:128, msl], qtok[:, t, ts(hb, Y)],
                            etb[:, t, msl],
                            start=(t == 0), stop=(t == NT - 1),
                            tile_position=(0, 64), skip_group_check=True)
                nc.vector.tensor_copy(ak[:, hp, :], akp[:])

                # E pass: E[m,n] = exp(beta k_m.q_n)  (reuses the ET slots)
                ea = sb.tile([P, NT, N], BF16, tag="ebig", bufs=3, name="ea")
                eb = sb.tile([P, NT, N], BF16, tag="ebig", bufs=3, name="eb")
                for t in range(NT):
                    pa = pbig("pa")
                    pb = pbig("pb")
                    for c in range(NCH):
                        nsl = ts(c, 512)
                        nc.tensor.matmul(
                            pa[:, nsl], kf[0:64, ts(t, P)], qf[0:64, nsl],
                            start=True, stop=True, skip_group_check=True)
                        nc.tensor.matmul(
                            pb[:, nsl], kf[64:128, ts(t, P)], qf[64:128, nsl],
                            start=True, stop=True, skip_group_check=True)
                    nc.scalar.activation(ea[:, t, :], pa[:], AF.Exp,
                                         scale=BETA)
                    nc.scalar.activation(eb[:, t, :], pb[:], AF.Exp,
                                         scale=BETA)

                # attnQ: aq[y,n] = sum_m K[m,y] E[m,n]   (col-packed heads)
                aqp = pbig("aqp")
                for c in range(NCH):
                    nsl = ts(c, 512)
                    for t in range(NT):
                        nc.tensor.matmul(
                            aqp[0:64, nsl], ktok[:, t, ts(ha, Y)],
                            ea[:, t, nsl],
                            start=(t == 0), stop=(t == NT - 1),
                            skip_group_check=True)
                        nc.tensor.matmul(
                            aqp[64:128, nsl], ktok[:, t, ts(hb, Y)],
                            eb[:, t, nsl],
                            start=(t == 0), stop=(t == NT - 1),
                            tile_position=(0, 64), skip_group_check=True)
                nc.vector.tensor_copy(aq[:, hp, :], aqp[:])

            # aq normalization: aq[y,n] *= 1/s_h[n]  (n on free axis).
            # rsf holds 1/s in (p,t)-permuted column order so the transposing
            # DMA is contiguous; AP views undo the permutation downstream.
            for h in range(H):
                hp, off = h // 2, 64 * (h % 2)
                rsf = sb.tile([1, N], F32, tag="rsf", bufs=1, name="rsf")
                nc.sync.dma_start(rsf[:], rs_all[:, h, :])
                rsv = rsf.rearrange("q (p t) -> q p t", t=NT)
                rrep = pbig("rrep")
                for c in range(NCH):
                    nsl = ts(c, 512)
                    rhs = rsv[:, :, 4 * c:4 * c + 4]
                    if off == 0:
                        nc.tensor.matmul(
                            rrep[0:64, nsl], ones_r[:1, :64], rhs,
                            start=True, stop=True, skip_group_check=True)
                    else:
                        nc.tensor.matmul(
                            rrep[64:128, nsl], ones_r[:1, :64], rhs,
                            start=True, stop=True, tile_position=(0, 64),
                            skip_group_check=True)
                    # rrep columns are (p, t)-ordered; view aq to match.
                    nc.vector.tensor_tensor(
                        aq[off:off + 64, hp, nsl]
                        .rearrange("y (t p) -> y p t", p=P),
                        aq[off:off + 64, hp, nsl]
                        .rearrange("y (t p) -> y p t", p=P),
                        rrep[off:off + 64, nsl]
                        .rearrange("y (p t) -> y p t", t=4),
                        ALU.mult)

            # ---------------- Phase D: gradient accumulation + x update
            for c in range(NCH):
                nsl = ts(c, 512)
                gbig = [pbig(f"gb{i}") for i in range(DS // 2)]
                gps = [gbig[d // 2][:, (d % 2) * 512:(d % 2) * 512 + 512]
                       for d in range(DS)]
                hbig = pbig("hbig")
                for dt in range(DS):
                    for s_ in range(DS):
                        nc.tensor.matmul(
                            gps[dt], wqFs[:, s_, ts(dt, P)], aq[:, s_, nsl],
                            start=(s_ == 0), stop=False, skip_group_check=True)
                    for s_ in range(DS):
                        nc.tensor.matmul(
                            gps[dt], wkFs[:, s_, ts(dt, P)], ak[:, s_, nsl],
                            start=False, stop=False, skip_group_check=True)
                for msp in range(MS // 2):
                    xit = sb.tile([P, DS, 2 * P], BF16, tag="xit", bufs=2,
                                  name="xit")
                    nc.sync.dma_start(xit[:], xiT_v[:, :, ts(msp, 2 * P)])
                    xis = sb.tile([P, 2, D], BF16, tag="xis", bufs=2,
                                  name="xis")
                    nc.sync.dma_start(xis[:], xiS_v[:, 2 * msp:2 * msp + 2, :])
                    for j in range(2):
                        ms = 2 * msp + j
                        hp_ = hbig[:, j * 512:j * 512 + 512]
                        for d in range(DS):
                            nc.tensor.matmul(
                                hp_, xit[:, d, ts(j, P)], gs[:, d, nsl],
                                start=(d == 0), stop=(d == DS - 1),
                                skip_group_check=True)
                        rt = sb.tile([P, 512], BF16, tag="rt", bufs=2,
                                     name="rt")
                        nc.scalar.activation(rt[:], hp_, AF.Relu)
                        for dt in range(DS):
                            nc.tensor.matmul(
                                gps[dt], xis[:, j, ts(dt, P)], rt[:],
                                start=False, stop=(ms == MS - 1),
                                skip_group_check=True)
                for dt in range(DS):
                    nc.vector.tensor_tensor(
                        xs[:, dt, nsl], xs[:, dt, nsl], gps[dt], ALU.add)

        if loop_mode == "fori" and steps > 1:
            # Final step unrolled: reads of state written inside a For_i from
            # after the loop are not dependency-tracked (observed to race), so
            # keep the loop-exit consumer chain in straight-line code.
            with tc.For_i(0, steps - 1, 1,
                          hint_engines=(ET_.PE, ET_.Activation, ET_.DVE,
                                        ET_.SP, ET_.Pool)):
                emit_step()
            emit_step()
        else:
            for _ in range(steps):
                emit_step()

        nc.sync.dma_start(
            xo_d.ap().rearrange("(o p) n -> p o n", p=P), xs[:])

    nc.compile()
    return nc


# ---------------------------------------------------------------- host side
def _prep_shared(ln_gamma, ln_beta, wq, wk, xi):
    bf = ml_dtypes.bfloat16
    wq_f = np.ascontiguousarray(wq.reshape(HY, D))
    wk_f = np.ascontiguousarray(wk.reshape(HY, D))
    return {
        "wqT": np.ascontiguousarray(wq_f.T).astype(bf),
        "wkT": np.ascontiguousarray(wk_f.T).astype(bf),
        "wqF": (ALPHA * wq_f).astype(bf),
        "wkF": (ALPHA * wk_f).astype(bf),
        "xiT": np.ascontiguousarray(xi.T).astype(bf),
        "xiS": (ALPHA * xi).astype(bf),
        "gamma": np.ascontiguousarray(ln_gamma, dtype=np.float32),
        "beta": np.ascontiguousarray(ln_beta, dtype=np.float32),
    }


def make_in_maps(x, ln_gamma, ln_beta, wq, wk, xi):
    shared = _prep_shared(np.asarray(ln_gamma), np.asarray(ln_beta),
                          np.asarray(wq), np.asarray(wk), np.asarray(xi))
    x = np.asarray(x, dtype=np.float32)
    maps = []
    for b in range(B):
        m = dict(shared)
        m["x"] = np.ascontiguousarray(x[b].T)
        maps.append(m)
    return maps


def get_executor(steps=STEPS, loop_mode="fori"):
    """Build+compile once; return (nc, run_fn). run_fn(in_maps) -> results
    list; repeated calls reuse the compiled PJRT executable."""
    key = (steps, loop_mode)
    with _lock:
        if key in _cache:
            return _cache[key]
    nc = build_nc(steps, loop_mode)

    import jax
    from jax.sharding import Mesh, PartitionSpec
    from jax.experimental.shard_map import shard_map
    from concourse import bass2jax

    bass2jax.install_neuronx_cc_hook()

    in_names, out_names, out_avals, zero_outs = [], [], [], []
    for alloc in nc.m.functions[0].allocations:
        if not isinstance(alloc, mybir.MemoryLocationSet):
            continue
        name = alloc.memorylocations[0].name
        if alloc.kind == "ExternalInput":
            in_names.append(name)
        elif alloc.kind == "ExternalOutput":
            out_names.append(name)
            shape = tuple(alloc.tensor_shape)
            dtype = mybir.dt.np(alloc.dtype)
            out_avals.append(jax.core.ShapedArray(shape, dtype))
            zero_outs.append(np.zeros(shape, dtype))
    partition_name = (nc.partition_id_tensor.name
                      if nc.partition_id_tensor else None)
    if partition_name is not None and partition_name in in_names:
        in_names.remove(partition_name)
    n_params = len(in_names)
    n_outs = len(out_avals)
    all_names = in_names + out_names
    if partition_name is not None:
        all_names = all_names + [partition_name]

    def _body(*args):
        operands = list(args)
        if partition_name is not None:
            operands.append(bass2jax.partition_id_tensor())
        outs = bass2jax._bass_exec_p.bind(
            *operands,
            out_avals=tuple(out_avals),
            in_names=tuple(all_names),
            out_names=tuple(out_names),
            lowering_input_output_aliases=(),
            sim_require_finite=True,
            sim_require_nnan=True,
            nc=nc,
        )
        return tuple(outs)

    devices = jax.devices()[:B]
    mesh = Mesh(np.asarray(devices), ("core",))
    sharded = jax.jit(
        shard_map(_body, mesh=mesh,
                  in_specs=(PartitionSpec("core"),) * (n_params + n_outs),
                  out_specs=(PartitionSpec("core"),) * n_outs,
                  check_rep=False),
        keep_unused=True,
    )

    def _concat(in_maps):
        per_core = [[np.asarray(m[nm]) for nm in in_names] for m in in_maps]
        concat_in = [
            np.concatenate([per_core[c][i] for c in range(B)], axis=0)
            for i in range(n_params)
        ]
        concat_zeros = [
            np.zeros((B * z.shape[0], *z.shape[1:]), z.dtype)
            for z in zero_outs
        ]
        return concat_in, concat_zeros

    def _unpack(out_arrs):
        out_arrs = [np.asarray(a) for a in out_arrs]
        return [
            {nm: out_arrs[i].reshape(B, *out_avals[i].shape)[c]
             for i, nm in enumerate(out_names)}
            for c in range(B)
        ]

    def run(in_maps):
        concat_in, concat_zeros = _concat(in_maps)
        return _unpack(sharded(*concat_in, *concat_zeros))

    def run_device(in_maps, reps=3):
        """Device-resident timing: transfer once, execute reps times.
        Returns (results, [per-call seconds])."""
        import time as _time
        from jax.sharding import NamedSharding
        concat_in, concat_zeros = _concat(in_maps)
        shd = NamedSharding(mesh, PartitionSpec("core"))
        dev_in = [jax.device_put(a, shd) for a in concat_in]
        dev_z = [jax.device_put(a, shd) for a in concat_zeros]
        out = sharded(*dev_in, *dev_z)
        jax.block_until_ready(out)
        times = []
        for _ in range(reps):
            t0 = _time.perf_counter()
            out = sharded(*dev_in, *dev_z)
            jax.block_until_ready(out)
            times.append(_time.perf_counter() - t0)
        return _unpack(out), times

    with _lock:
        _cache[key] = (nc, run, run_device)
    return nc, run, run_device


def kernel(x, ln_gamma, ln_beta, wq, wk, xi):
    _, run, _ = get_executor()
    in_maps = make_in_maps(x, ln_gamma, ln_beta, wq, wk, xi)
    results = run(in_maps)
    out = np.stack([results[b]["xout"].T for b in range(B)])
    return np.ascontiguousarray(out, dtype=np.float32)


# revision 26
# speedup vs baseline: 37.3955x; 1.0126x over previous
"""Trainium2 Bass kernel for the Energy Transformer problem.

Sharding: data-parallel over batch B=8 — one batch element per NeuronCore,
zero collectives.  All state stays SBUF-resident across the 12 descent steps;
only the Hopfield memory matrix (xi) is streamed from HBM during the gradient
phase.

Per-core layout convention: feature-major ("F layout") — tensors of shape
[feat, tokens] stored as SBUF [128, feat//128, tokens] with feat on partitions.

Per step (analytic gradient of the energy, derived by hand and validated
against jax.grad):
  g      = LayerNorm(x)                        (stats via ones-matmuls)
  Q,K    = Wq g, Wk g                          (both [hy,n] and [n,hy] layouts)
  per head h:
    ET[n,m] = exp(beta * q_n . k_m)            (accum_out gives s[n] for free)
    E [m,n] = exp(beta * k_m . q_n)
    aq[y,n] = sum_m K[m,y] E[m,n]              (attn-Q term, normalized by 1/s)
    ak[y,n] = sum_n' ET[n',m] (Q[n',y]/s[n'])  (attn-K term)
  hid[m,n] = Xi g ;  r = relu(hid)
  x += alpha * (Wq^T aq + Wk^T ak + Xi^T r)    (one PSUM accumulation chain)
"""

import os
import threading

import numpy as np
import ml_dtypes

import concourse.bass as bass
import concourse.mybir as mybir
import concourse.tile as tile
from concourse import bacc
from concourse.bass import ts

# ---------------------------------------------------------------- constants
B, N, D = 8, 1024, 768
H, Y = 12, 64
HY = H * Y          # 768
M = 3072
STEPS = 12
ALPHA = 0.1
BETA = 1.0 / float(np.sqrt(Y))
EPS = 1e-5

P = 128
DS = D // P         # 6  d-subtiles
NT = N // P         # 8  token tiles
NCH = N // 512      # 2  512-wide free chunks
MS = M // P         # 24 memory subtiles
NPAIR = H // 2      # 6  head pairs

F32 = mybir.dt.float32
BF16 = mybir.dt.bfloat16
AF = mybir.ActivationFunctionType
ALU = mybir.AluOpType
AX = mybir.AxisListType
ET_ = mybir.EngineType

_lock = threading.Lock()
_cache = {}


# ---------------------------------------------------------------- builder
def build_nc(steps=STEPS, loop_mode="fori"):
    """Build the per-core Bass kernel. Same NEFF runs SPMD on all 8 cores."""
    # Allow using the full 208 KiB/partition of SBUF (stale default is 192).
    try:
        from concourse import tile_utils
        tile_utils.max_sbuf_usage = 208 * 1024
    except Exception:
        pass

    nc = bacc.Bacc("TRN2", target_bir_lowering=False, debug=False)

    # DRAM I/O (per core). Weight tensors are pre-transposed/scaled on host.
    x_d = nc.dram_tensor("x", [D, N], F32, kind="ExternalInput")
    wqT_d = nc.dram_tensor("wqT", [D, HY], BF16, kind="ExternalInput")
    wkT_d = nc.dram_tensor("wkT", [D, HY], BF16, kind="ExternalInput")
    wqF_d = nc.dram_tensor("wqF", [HY, D], BF16, kind="ExternalInput")
    wkF_d = nc.dram_tensor("wkF", [HY, D], BF16, kind="ExternalInput")
    xiT_d = nc.dram_tensor("xiT", [D, M], BF16, kind="ExternalInput")
    xiS_d = nc.dram_tensor("xiS", [M, D], BF16, kind="ExternalInput")
    gam_d = nc.dram_tensor("gamma", [D], F32, kind="ExternalInput")
    bet_d = nc.dram_tensor("beta", [D], F32, kind="ExternalInput")
    xo_d = nc.dram_tensor("xout", [D, N], F32, kind="ExternalOutput")

    # Persistent SBUF state.
    xs = nc.alloc_sbuf_tensor("xs", [P, DS, N], F32).ap()
    gs = nc.alloc_sbuf_tensor("gs", [P, DS, N], BF16).ap()
    wqTs = nc.alloc_sbuf_tensor("wqTs", [P, DS, HY], BF16).ap()
    wkTs = nc.alloc_sbuf_tensor("wkTs", [P, DS, HY], BF16).ap()
    wqFs = nc.alloc_sbuf_tensor("wqFs", [P, DS, D], BF16).ap()
    wkFs = nc.alloc_sbuf_tensor("wkFs", [P, DS, D], BF16).ap()
    qtok = nc.alloc_sbuf_tensor("qtok", [P, NT, HY], BF16).ap()
    ktok = nc.alloc_sbuf_tensor("ktok", [P, NT, HY], BF16).ap()
    aq = nc.alloc_sbuf_tensor("aq", [P, DS, N], BF16).ap()
    ak = nc.alloc_sbuf_tensor("ak", [P, DS, N], BF16).ap()
    gam_s = nc.alloc_sbuf_tensor("gam_s", [P, DS], F32).ap()
    bet_s = nc.alloc_sbuf_tensor("bet_s", [P, DS], F32).ap()
    ones_c = nc.alloc_sbuf_tensor("ones_c", [P, 1], F32).ap()   # lhsT for sums
    ones_r = nc.alloc_sbuf_tensor("ones_r", [1, P], F32).ap()   # lhsT for bcast
    eps_c = nc.alloc_sbuf_tensor("eps_c", [1, 1], F32).ap()
    rs_all = nc.alloc_sbuf_tensor("rs_all", [P, H, NT], F32).ap()

    from contextlib import ExitStack
    with tile.TileContext(nc) as tc, ExitStack() as stack:
        sb = stack.enter_context(tc.tile_pool(name="sb", bufs=2))
        psum = stack.enter_context(
            tc.tile_pool(name="psum", bufs=4, space="PSUM"))

        def pbig(name):
            return psum.tile([P, 1024], F32, tag="pw", bufs=2, name=name)

        def psmall(name):
            return psum.tile([P, 512], F32, tag="ps", bufs=4, name=name)

        # ---- one-time loads
        nc.gpsimd.memset(ones_c[:], 1.0)
        nc.gpsimd.memset(ones_r[:], 1.0)
        nc.gpsimd.memset(eps_c[:], EPS)
        nc.sync.dma_start(xs[:], x_d.ap().rearrange("(o p) n -> p o n", p=P))
        nc.sync.dma_start(wqTs[:], wqT_d.ap().rearrange("(o p) h -> p o h", p=P))
        nc.sync.dma_start(wkTs[:], wkT_d.ap().rearrange("(o p) h -> p o h", p=P))
        nc.sync.dma_start(wqFs[:], wqF_d.ap().rearrange("(o p) d -> p o d", p=P))
        nc.sync.dma_start(wkFs[:], wkF_d.ap().rearrange("(o p) d -> p o d", p=P))
        with nc.allow_non_contiguous_dma(reason="tiny 768-elem transposes"):
            nc.sync.dma_start(gam_s[:],
                              gam_d.ap().rearrange("(o p) -> p o", p=P))
            nc.sync.dma_start(bet_s[:],
                              bet_d.ap().rearrange("(o p) -> p o", p=P))

        xiT_v = xiT_d.ap().rearrange("(o p) m -> p o m", p=P)
        xiS_v = xiS_d.ap().rearrange("(o p) d -> p o d", p=P)

        def emit_step():
            # ---------------- Phase A: LayerNorm -> gs (bf16)
            for c in range(NCH):
                nsl = ts(c, 512)
                s1p = psmall("s1p")[:1, :]
                s2p = psmall("s2p")[:1, :]
                for d in range(DS):
                    x2t = sb.tile([P, 512], F32, tag="x2", bufs=1, name="x2t")
                    nc.vector.tensor_tensor(
                        x2t[:], xs[:, d, nsl], xs[:, d, nsl], ALU.mult)
                    nc.tensor.matmul(
                        s1p, ones_c[:], xs[:, d, nsl],
                        start=(d == 0), stop=(d == DS - 1),
                        skip_group_check=True)
                    nc.tensor.matmul(
                        s2p, ones_c[:], x2t[:],
                        start=(d == 0), stop=(d == DS - 1),
                        skip_group_check=True)
                mu = sb.tile([1, 512], F32, tag="mu", bufs=1, name="mu")
                nc.vector.tensor_scalar_mul(mu[:], s1p, 1.0 / D)
                var = sb.tile([1, 512], F32, tag="var", bufs=1, name="var")
                nc.vector.tensor_scalar_mul(var[:], s2p, 1.0 / D)
                musq = sb.tile([1, 512], F32, tag="musq", bufs=1, name="musq")
                nc.vector.tensor_tensor(musq[:], mu[:], mu[:], ALU.mult)
                nc.vector.tensor_tensor(var[:], var[:], musq[:], ALU.subtract)
                # rstd = exp(-0.5*ln(var+eps))
                rstd = sb.tile([1, 512], F32, tag="rstd", bufs=1, name="rstd")
                nc.scalar.activation(rstd[:], var[:], AF.Ln, bias=eps_c[:])
                nc.scalar.activation(rstd[:], rstd[:], AF.Exp, scale=-0.5)
                # replicate mu/rstd across partitions via K=1 matmul
                mur = psmall("mur")
                rsr = psmall("rsr")
                nc.tensor.matmul(mur[:], ones_r[:1, :], mu[:],
                                 start=True, stop=True)
                nc.tensor.matmul(rsr[:], ones_r[:1, :], rstd[:],
                                 start=True, stop=True)
                for d in range(DS):
                    tt = sb.tile([P, 512], F32, tag="lnt", bufs=1, name="tt")
                    nc.vector.tensor_tensor(
                        tt[:], xs[:, d, nsl], mur[:], ALU.subtract)
                    nc.vector.tensor_tensor(tt[:], tt[:], rsr[:], ALU.mult)
                    nc.vector.tensor_scalar(
                        gs[:, d, nsl], tt[:],
                        gam_s[:, d:d + 1], bet_s[:, d:d + 1],
                        ALU.mult, ALU.add)

                # token-layout Q/K projections for this chunk's token tiles
                for t in range(4 * c, 4 * c + 4):
                    qtp = pbig("qtp")
                    ktp = pbig("ktp")
                    for c0, cw in ((0, 512), (512, 256)):
                        for d in range(DS):
                            nc.tensor.matmul(
                                qtp[:, c0:c0 + cw], gs[:, d, ts(t, P)],
                                wqTs[:, d, c0:c0 + cw],
                                start=(d == 0), stop=(d == DS - 1),
                                skip_group_check=True)
                            nc.tensor.matmul(
                                ktp[:, c0:c0 + cw], gs[:, d, ts(t, P)],
                                wkTs[:, d, c0:c0 + cw],
                                start=(d == 0), stop=(d == DS - 1),
                                skip_group_check=True)
                    nc.scalar.copy(qtok[:, t, :], qtp[:, :HY])
                    nc.scalar.copy(ktok[:, t, :], ktp[:, :HY])


            # ---------------- Phase C: attention, software-pipelined pairs.
            # E/ET matrices live in six 8KB half-tiles (t 0-3 / 4-7 per
            # head). attnQ of pair hp-1 interleaves with the ET matmuls of
            # pair hp so PE fills the gaps behind the ACT exp wavefront.
            def ehalf(name):
                return sb.tile([P, NT // 2, N], BF16, tag="eb2", bufs=6,
                               name=name)

            def attnq_steps(hp_, ea_, eb_):
                """Generator: 8 yields of per-t attnQ matmuls, then the
                copy-back + 1/s normalization tail on the 9th next()."""
                ha_, hb_ = 2 * hp_, 2 * hp_ + 1
                aqps = (psmall("aqp0"), psmall("aqp1"))
                for t in range(NT):
                    for c in range(NCH):
                        nsl = ts(c, 512)
                        nc.tensor.matmul(
                            aqps[c][0:64, :], ktok[:, t, ts(ha_, Y)],
                            ea_[t // 4][:, t % 4, nsl],
                            start=(t == 0), stop=(t == NT - 1),
                            skip_group_check=True)
                        nc.tensor.matmul(
                            aqps[c][64:128, :], ktok[:, t, ts(hb_, Y)],
                            eb_[t // 4][:, t % 4, nsl],
                            start=(t == 0), stop=(t == NT - 1),
                            tile_position=(0, 64), skip_group_check=True)
                    yield
                for c in range(NCH):
                    nc.vector.tensor_copy(aq[:, hp_, ts(c, 512)], aqps[c][:])
                # normalize: aq[y,n] *= 1/s_h[n] (n on the free axis). rsf
                # holds 1/s in (p,t)-permuted order so the transposing DMA
                # is contiguous; AP views undo the permutation.
                for h in (ha_, hb_):
                    off = 64 * (h % 2)
                    rsf = sb.tile([1, N], F32, tag="rsf", bufs=1, name="rsf")
                    nc.sync.dma_start(rsf[:], rs_all[:, h, :])
                    rsv = rsf.rearrange("q (p t) -> q p t", t=NT)
                    for c in range(NCH):
                        nsl = ts(c, 512)
                        rhs = rsv[:, :, 4 * c:4 * c + 4]
                        rrep = psmall("rrep")
                        if off == 0:
                            nc.tensor.matmul(
                                rrep[0:64, :], ones_r[:1, :64], rhs,
                                start=True, stop=True)
                        else:
                            nc.tensor.matmul(
                                rrep[64:128, :], ones_r[:1, :64], rhs,
                                start=True, stop=True,
                                tile_position=(0, 64))
                        nc.vector.tensor_tensor(
                            aq[off:off + 64, hp_, nsl]
                            .rearrange("y (t p) -> y p t", p=P),
                            aq[off:off + 64, hp_, nsl]
                            .rearrange("y (t p) -> y p t", p=P),
                            rrep[off:off + 64, :]
                            .rearrange("y (p t) -> y p t", t=4),
                            ALU.mult)
                while True:
                    yield

            prev = None
            for hp in range(NPAIR):
                ha, hb = 2 * hp, 2 * hp + 1
                # F-layout Q/K for this pair's 128 hy rows
                qf = sb.tile([P, N], BF16, tag="qf", bufs=2, name="qf")
                kf = sb.tile([P, N], BF16, tag="kf", bufs=2, name="kf")
                qp = pbig("qp")
                kp = pbig("kp")
                for c in range(NCH):
                    nsl = ts(c, 512)
                    for d in range(DS):
                        nc.tensor.matmul(
                            qp[:, nsl], wqTs[:, d, ts(hp, P)], gs[:, d, nsl],
                            start=(d == 0), stop=(d == DS - 1),
                            skip_group_check=True)
                        nc.tensor.matmul(
                            kp[:, nsl], wkTs[:, d, ts(hp, P)], gs[:, d, nsl],
                            start=(d == 0), stop=(d == DS - 1),
                            skip_group_check=True)
                nc.scalar.copy(qf[:], qp[:])
                nc.scalar.copy(kf[:], kp[:])

                # ET pass: ET[n,m] = exp(beta q_n.k_m); accum_out -> s[n].
                # Interleaved with attnQ of the previous pair.
                eta, etb = [None, None], [None, None]
                sca = sb.tile([P, NT], F32, tag="sca", bufs=2, name="sca")
                scb = sb.tile([P, NT], F32, tag="scb", bufs=2, name="scb")
                for t in range(NT):
                    if t % 4 == 0:
                        eta[t // 4] = ehalf("eta")
                        etb[t // 4] = ehalf("etb")
                    pa = pbig("pa")
                    pb = pbig("pb")
                    for c in range(NCH):
                        msl = ts(c, 512)
                        nc.tensor.matmul(
                            pa[:, msl], qf[0:64, ts(t, P)], kf[0:64, msl],
                            start=True, stop=True, skip_group_check=True)
                        nc.tensor.matmul(
                            pb[:, msl], qf[64:128, ts(t, P)], kf[64:128, msl],
                            start=True, stop=True, skip_group_check=True)
                    if prev is not None:
                        next(prev)
                    nc.scalar.activation(
                        eta[t // 4][:, t % 4, :], pa[:], AF.Exp, scale=BETA,
                        accum_out=sca[:, t:t + 1])
                    nc.scalar.activation(
                        etb[t // 4][:, t % 4, :], pb[:], AF.Exp, scale=BETA,
                        accum_out=scb[:, t:t + 1])
                if prev is not None:
                    next(prev)   # attnQ(hp-1) copy-back + normalization
                    prev = None

                # s -> 1/s ; Q' = Q * (1/s)  (per-token scale, partition dim)
                for h, sc in ((ha, sca), (hb, scb)):
                    nc.vector.reciprocal(rs_all[:, h, :], sc[:])
                    nc.vector.tensor_tensor(
                        qtok[:, :, ts(h, Y)], qtok[:, :, ts(h, Y)],
                        rs_all[:, h, :, None].to_broadcast([P, NT, Y]),
                        ALU.mult)

                # E pass (E[m,n] = exp(beta k_m.q_n)) interleaved with attnK
                # (ak[y,m] = sum_n ET[n,m] Q'[n,y], col-packed heads) so PE
                # fills the gaps behind the exp wavefront.
                ea, eb = [None, None], [None, None]
                akps = (psmall("akp0"), psmall("akp1"))
                for t in range(NT):
                    if t % 4 == 0:
                        ea[t // 4] = ehalf("ea")
                        eb[t // 4] = ehalf("eb")
                    pa = pbig("pa")
                    pb = pbig("pb")
                    for c in range(NCH):
                        nsl = ts(c, 512)
                        nc.tensor.matmul(
                            pa[:, nsl], kf[0:64, ts(t, P)], qf[0:64, nsl],
                            start=True, stop=True, skip_group_check=True)
                        nc.tensor.matmul(
                            pb[:, nsl], kf[64:128, ts(t, P)], qf[64:128, nsl],
                            start=True, stop=True, skip_group_check=True)
                    for c in range(NCH):
                        msl = ts(c, 512)
                        nc.tensor.matmul(
                            akps[c][0:64, :], qtok[:, t, ts(ha, Y)],
                            eta[t // 4][:, t % 4, msl],
                            start=(t == 0), stop=(t == NT - 1),
                            skip_group_check=True)
                        nc.tensor.matmul(
                            akps[c][64:128, :], qtok[:, t, ts(hb, Y)],
                            etb[t // 4][:, t % 4, msl],
                            start=(t == 0), stop=(t == NT - 1),
                            tile_position=(0, 64), skip_group_check=True)
                    nc.scalar.activation(ea[t // 4][:, t % 4, :], pa[:],
                                         AF.Exp, scale=BETA)
                    nc.scalar.activation(eb[t // 4][:, t % 4, :], pb[:],
                                         AF.Exp, scale=BETA)
                for c in range(NCH):
                    nc.vector.tensor_copy(ak[:, hp, ts(c, 512)], akps[c][:])
                prev = attnq_steps(hp, ea, eb)
            for _ in range(NT + 1):
                next(prev)
            prev = None

            # ---------------- Phase D: gradient accumulation + x update
            for c in range(NCH):
                nsl = ts(c, 512)
                gw = pbig("gw")
                gps = [psmall(f"gp{d}") for d in range(4)] + \
                      [gw[:, 0:512], gw[:, 512:1024]]
                hbig = pbig("hbig")
                for msp in range(MS // 2):
                    xit = sb.tile([P, DS, 2 * P], BF16, tag="xit", bufs=2,
                                  name="xit")
                    nc.sync.dma_start(xit[:], xiT_v[:, :, ts(msp, 2 * P)])
                    xis = sb.tile([P, 2, D], BF16, tag="xis", bufs=2,
                                  name="xis")
                    nc.sync.dma_start(xis[:], xiS_v[:, 2 * msp:2 * msp + 2, :])
                    for j in range(2):
                        ms = 2 * msp + j
                        hp_ = hbig[:, j * 512:j * 512 + 512]
                        for d in range(DS):
                            nc.tensor.matmul(
                                hp_, xit[:, d, ts(j, P)], gs[:, d, nsl],
                                start=(d == 0), stop=(d == DS - 1),
                                skip_group_check=True)
                        rt = sb.tile([P, 512], BF16, tag="rt", bufs=2,
                                     name="rt")
                        nc.scalar.activation(rt[:], hp_, AF.Relu)
                        for dt in range(DS):
                            nc.tensor.matmul(
                                gps[dt], xis[:, j, ts(dt, P)], rt[:],
                                start=(ms == 0), stop=False,
                                skip_group_check=True)
                for dt in range(DS):
                    for s_ in range(DS):
                        nc.tensor.matmul(
                            gps[dt], wqFs[:, s_, ts(dt, P)], aq[:, s_, nsl],
                            start=False, stop=False, skip_group_check=True)
                    for s_ in range(DS):
                        nc.tensor.matmul(
                            gps[dt], wkFs[:, s_, ts(dt, P)], ak[:, s_, nsl],
                            start=False, stop=(s_ == DS - 1),
                            skip_group_check=True)
                for dt in range(DS):
                    nc.vector.tensor_tensor(
                        xs[:, dt, nsl], xs[:, dt, nsl], gps[dt], ALU.add)

        if loop_mode == "fori" and steps > 1:
            # Final step unrolled: reads of state written inside a For_i from
            # after the loop are not dependency-tracked (observed to race), so
            # keep the loop-exit consumer chain in straight-line code.
            with tc.For_i(0, steps - 1, 1,
                          hint_engines=(ET_.PE, ET_.Activation, ET_.DVE,
                                        ET_.SP, ET_.Pool)):
                emit_step()
            emit_step()
        else:
            for _ in range(steps):
                emit_step()

        nc.sync.dma_start(
            xo_d.ap().rearrange("(o p) n -> p o n", p=P), xs[:])

    nc.compile()
    return nc


# ---------------------------------------------------------------- host side
def _prep_shared(ln_gamma, ln_beta, wq, wk, xi):
    bf = ml_dtypes.bfloat16
    wq_f = np.ascontiguousarray(wq.reshape(HY, D))
    wk_f = np.ascontiguousarray(wk.reshape(HY, D))
    return {
        "wqT": np.ascontiguousarray(wq_f.T).astype(bf),
        "wkT": np.ascontiguousarray(wk_f.T).astype(bf),
        "wqF": (ALPHA * wq_f).astype(bf),
        "wkF": (ALPHA * wk_f).astype(bf),
        "xiT": np.ascontiguousarray(xi.T).astype(bf),
        "xiS": (ALPHA * xi).astype(bf),
        "gamma": np.ascontiguousarray(ln_gamma, dtype=np.float32),
        "beta": np.ascontiguousarray(ln_beta, dtype=np.float32),
    }


def make_in_maps(x, ln_gamma, ln_beta, wq, wk, xi):
    shared = _prep_shared(np.asarray(ln_gamma), np.asarray(ln_beta),
                          np.asarray(wq), np.asarray(wk), np.asarray(xi))
    x = np.asarray(x, dtype=np.float32)
    maps = []
    for b in range(B):
        m = dict(shared)
        m["x"] = np.ascontiguousarray(x[b].T)
        maps.append(m)
    return maps


def get_executor(steps=STEPS, loop_mode="fori"):
    """Build+compile once; return (nc, run_fn). run_fn(in_maps) -> results
    list; repeated calls reuse the compiled PJRT executable."""
    key = (steps, loop_mode)
    with _lock:
        if key in _cache:
            return _cache[key]
    nc = build_nc(steps, loop_mode)

    import jax
    from jax.sharding import Mesh, PartitionSpec
    from jax.experimental.shard_map import shard_map
    from concourse import bass2jax

    bass2jax.install_neuronx_cc_hook()

    in_names, out_names, out_avals, zero_outs = [], [], [], []
    for alloc in nc.m.functions[0].allocations:
        if not isinstance(alloc, mybir.MemoryLocationSet):
            continue
        name = alloc.memorylocations[0].name
        if alloc.kind == "ExternalInput":
            in_names.append(name)
        elif alloc.kind == "ExternalOutput":
            out_names.append(name)
            shape = tuple(alloc.tensor_shape)
            dtype = mybir.dt.np(alloc.dtype)
            out_avals.append(jax.core.ShapedArray(shape, dtype))
            zero_outs.append(np.zeros(shape, dtype))
    partition_name = (nc.partition_id_tensor.name
                      if nc.partition_id_tensor else None)
    if partition_name is not None and partition_name in in_names:
        in_names.remove(partition_name)
    n_params = len(in_names)
    n_outs = len(out_avals)
    all_names = in_names + out_names
    if partition_name is not None:
        all_names = all_names + [partition_name]

    def _body(*args):
        operands = list(args)
        if partition_name is not None:
            operands.append(bass2jax.partition_id_tensor())
        outs = bass2jax._bass_exec_p.bind(
            *operands,
            out_avals=tuple(out_avals),
            in_names=tuple(all_names),
            out_names=tuple(out_names),
            lowering_input_output_aliases=(),
            sim_require_finite=True,
            sim_require_nnan=True,
            nc=nc,
        )
        return tuple(outs)

    devices = jax.devices()[:B]
    mesh = Mesh(np.asarray(devices), ("core",))
    sharded = jax.jit(
        shard_map(_body, mesh=mesh,
                  in_specs=(PartitionSpec("core"),) * (n_params + n_outs),
                  out_specs=(PartitionSpec("core"),) * n_outs,
                  check_rep=False),
        keep_unused=True,
    )

    def _concat(in_maps):
        per_core = [[np.asarray(m[nm]) for nm in in_names] for m in in_maps]
        concat_in = [
            np.concatenate([per_core[c][i] for c in range(B)], axis=0)
            for i in range(n_params)
        ]
        concat_zeros = [
            np.zeros((B * z.shape[0], *z.shape[1:]), z.dtype)
            for z in zero_outs
        ]
        return concat_in, concat_zeros

    def _unpack(out_arrs):
        out_arrs = [np.asarray(a) for a in out_arrs]
        return [
            {nm: out_arrs[i].reshape(B, *out_avals[i].shape)[c]
             for i, nm in enumerate(out_names)}
            for c in range(B)
        ]

    def run(in_maps):
        concat_in, concat_zeros = _concat(in_maps)
        return _unpack(sharded(*concat_in, *concat_zeros))

    def run_device(in_maps, reps=3):
        """Device-resident timing: transfer once, execute reps times.
        Returns (results, [per-call seconds])."""
        import time as _time
        from jax.sharding import NamedSharding
        concat_in, concat_zeros = _concat(in_maps)
        shd = NamedSharding(mesh, PartitionSpec("core"))
        dev_in = [jax.device_put(a, shd) for a in concat_in]
        dev_z = [jax.device_put(a, shd) for a in concat_zeros]
        out = sharded(*dev_in, *dev_z)
        jax.block_until_ready(out)
        times = []
        for _ in range(reps):
            t0 = _time.perf_counter()
            out = sharded(*dev_in, *dev_z)
            jax.block_until_ready(out)
            times.append(_time.perf_counter() - t0)
        return _unpack(out), times

    def make_chain_runner(k):
        """jit that executes the NEFF k times, chaining xout -> x.
        Wall-time difference between k values isolates pure exec time."""
        xi_idx = in_names.index("x")

        def _chain(*args):
            ins = list(args[:n_params])
            zs = list(args[n_params:])
            for _ in range(k):
                outs = _body(*ins, *zs)
                ins[xi_idx] = outs[out_names.index("xout")]
            return outs

        return jax.jit(
            shard_map(_chain, mesh=mesh,
                      in_specs=(PartitionSpec("core"),) * (n_params + n_outs),
                      out_specs=(PartitionSpec("core"),) * n_outs,
                      check_rep=False),
            keep_unused=True,
        )

    def run_chained(in_maps, k, reps=3):
        import time as _time
        from jax.sharding import NamedSharding
        concat_in, concat_zeros = _concat(in_maps)
        shd = NamedSharding(mesh, PartitionSpec("core"))
        dev_in = [jax.device_put(a, shd) for a in concat_in]
        dev_z = [jax.device_put(a, shd) for a in concat_zeros]
        fn = make_chain_runner(k)
        out = fn(*dev_in, *dev_z)
        jax.block_until_ready(out)
        times = []
        for _ in range(reps):
            t0 = _time.perf_counter()
            out = fn(*dev_in, *dev_z)
            jax.block_until_ready(out)
            times.append(_time.perf_counter() - t0)
        return _unpack(out), times

    with _lock:
        _cache[key] = (nc, run, run_device, run_chained)
    return nc, run, run_device, run_chained


def kernel(x, ln_gamma, ln_beta, wq, wk, xi):
    _, run = get_executor()[:2]
    in_maps = make_in_maps(x, ln_gamma, ln_beta, wq, wk, xi)
    results = run(in_maps)
    out = np.stack([results[b]["xout"].T for b in range(B)])
    return np.ascontiguousarray(out, dtype=np.float32)
